# revision 1
# baseline (speedup 1.0000x reference)
"""Trainium2 Bass kernel for nn_EntityCell (scatter_memory).

Math (per batch row b, entity e):
    gates = sigmoid(sum_d(s * (h + k)))              [B, E]
    h_tilda = tanh(h @ U + k @ V + (s @ W)[:, None]) [B, E, D]
    updated = h + gates[:, :, None] * h_tilda
    out = updated / sqrt(max(sum_d(updated^2), 1e-12))

Sharding: pure data parallel over the batch dim across 8 NeuronCores.

Per-core dataflow (B_loc=1024 rows; 4 load-pairs of 256 rows, each processed
as two 128-row compute chunks):
  - HWDGE DMA loads fp32 row-pairs; ScalarE casts each pair to fp16 in one op.
  - One DMA xbar transpose per tensor per chunk produces d-major fp16 tiles.
  - GpSimd computes (hT+kT) and *sT (TT-class ops never contend with DVE).
  - PE: per-entity matmuls hT_e@U + kT_e@V + sT@W accumulated in PSUM
    (fp16 in, fp32 accum); gate reduction via lhsT=t2T_e, rhs=ones.
  - ScalarE: tanh on 512-wide PSUM tiles -> fp16; sigmoid for gates.
  - VectorE: fused scalar_tensor_tensor update u = g*h_tilda + h, bn_stats
    based sum(u^2), Newton rsqrt (bit-trick seed), final scale to fp32.
"""

import numpy as np
from contextlib import nullcontext as _nullctx

B, E, D = 8192, 20, 128
N_CORES = 8
B_LOC = B // N_CORES
CHUNK = 128
N_PAIRS = B_LOC // (2 * CHUNK)
EG = 4  # entities per psum group (4*128 fp32 = one 2KB PSUM bank)

_CACHE = {}


def _build_nc(reps=1, loop_n=None, bf_bufs=2, tr_bufs=3, newton_iters=2,
              scale_on_act=0, ablate='full', store16=False, scale_ts=True,
              psm_bufs=7, psg_bufs=1, sm_bufs=4, k_pe_tr=True, h_pe_tr=False, psk_bufs=2, h_evac_act=False):
    import concourse.tile as tile
    from concourse import bacc, mybir
    from concourse.masks import make_identity
    from contextlib import ExitStack

    fp32 = mybir.dt.float32
    fp16 = mybir.dt.float16
    int32 = mybir.dt.int32
    AF = mybir.ActivationFunctionType
    OP = mybir.AluOpType

    nc = bacc.Bacc("TRN2", target_bir_lowering=False, debug=False)
    enc_d = nc.declare_dram_parameter("enc", [B_LOC, D], fp32, isOutput=False)
    prev_d = nc.declare_dram_parameter("prev", [B_LOC, E, D], fp32, isOutput=False)
    keys_d = nc.declare_dram_parameter("keys", [B_LOC, E, D], fp32, isOutput=False)
    u_d = nc.declare_dram_parameter("U", [D, D], fp32, isOutput=False)
    v_d = nc.declare_dram_parameter("V", [D, D], fp32, isOutput=False)
    w_d = nc.declare_dram_parameter("W", [D, D], fp32, isOutput=False)
    out_d = nc.declare_dram_parameter("out", [B_LOC, E, D], fp32, isOutput=True)

    # DRAM views: 256-row blocks as [pair_idx, partition, 2*E*D]
    prev_v = prev_d[:].rearrange("(n two p) e d -> n p two (e d)", two=2, p=CHUNK)
    keys_v = keys_d[:].rearrange("(n two p) e d -> n p two (e d)", two=2, p=CHUNK)
    enc_v = enc_d[:].rearrange("(n two p) d -> n p two d", two=2, p=CHUNK)
    out_v = out_d[:].rearrange("(n p) e d -> n p (e d)", p=CHUNK)

    with ExitStack() as ctx:
        tc = ctx.enter_context(tile.TileContext(nc))
        const_pool = ctx.enter_context(tc.tile_pool(name="const", bufs=1))
        io_pool = ctx.enter_context(tc.tile_pool(name="io", bufs=2))
        bf_pool = ctx.enter_context(tc.tile_pool(name="bf", bufs=bf_bufs))
        tr_pool = ctx.enter_context(tc.tile_pool(name="tr", bufs=tr_bufs))
        sm_pool = ctx.enter_context(tc.tile_pool(name="sm", bufs=sm_bufs))
        if k_pe_tr and psm_bufs > 5:
            psm_bufs = 5
        psm_pool = ctx.enter_context(tc.tile_pool(name="psm", bufs=psm_bufs, space="PSUM"))
        psg_pool = ctx.enter_context(tc.tile_pool(name="psg", bufs=psg_bufs, space="PSUM"))
        psk_pool = (
            ctx.enter_context(tc.tile_pool(name="psk", bufs=psk_bufs, space="PSUM"))
            if (k_pe_tr or h_pe_tr)
            else None
        )

        # ---- constants ----
        u32c = const_pool.tile([D, D], fp32)
        v32c = const_pool.tile([D, D], fp32)
        w32c = const_pool.tile([D, D], fp32)
        nc.sync.dma_start(u32c[:], u_d[:])
        nc.sync.dma_start(v32c[:], v_d[:])
        nc.sync.dma_start(w32c[:], w_d[:])
        u16c = const_pool.tile([D, D], fp16)
        v16c = const_pool.tile([D, D], fp16)
        w16c = const_pool.tile([D, D], fp16)
        nc.scalar.copy(u16c[:], u32c[:])
        nc.scalar.copy(v16c[:], v32c[:])
        nc.scalar.copy(w16c[:], w32c[:])
        ones16 = const_pool.tile([D, 1], fp16)
        nc.gpsimd.memset(ones16[:], 1.0)
        magic = const_pool.tile([CHUNK, E], int32)
        nc.gpsimd.memset(magic[:], 0x5F3759DF)
        if k_pe_tr or h_pe_tr:
            ident16 = const_pool.tile([D, D], fp16)
            make_identity(nc, ident16[:])

        loop_cm = (
            tc.For_i(0, loop_n, 1, hint_engines=tuple(mybir.ALL_ENGINES))
            if loop_n is not None
            else _nullctx()
        )
        with loop_cm:
         for cp in range(N_PAIRS * reps):
             n = cp % N_PAIRS
             # ---- paired loads (256 rows -> [128, 2, E, D]) ----
             h32p = io_pool.tile([CHUNK, 2, E, D], fp32, name="h32p")
             nc.sync.dma_start(h32p[:].rearrange("p a e d -> p a (e d)"), prev_v[n])
             k32p = io_pool.tile([CHUNK, 2, E, D], fp32, name="k32p")
             nc.sync.dma_start(k32p[:].rearrange("p a e d -> p a (e d)"), keys_v[n])
             s32p = io_pool.tile([CHUNK, 2, D], fp32, name="s32p")
             nc.sync.dma_start(s32p[:], enc_v[n])

             if ablate == 'dma':
                 for half in range(2):
                     nc.sync.dma_start(
                         out=out_v[2 * n + half],
                         in_=h32p[:, half].rearrange("p e d -> p (e d)"),
                     )
                 continue
             # ---- casts to fp16 (ScalarE), one op per pair ----
             h16p = bf_pool.tile([CHUNK, 2, E, D], fp16, name="h16p")
             nc.scalar.copy(h16p[:], h32p[:])
             k16p = bf_pool.tile([CHUNK, 2, E, D], fp16, name="k16p")
             nc.scalar.copy(k16p[:], k32p[:])
             s16p = bf_pool.tile([CHUNK, 2, D], fp16, name="s16p")
             nc.scalar.copy(s16p[:], s32p[:])

             for half in range(2):
                 c = 2 * n + half
                 h16 = h16p[:, half]
                 k16 = k16p[:, half]
                 s16 = s16p[:, half]

                 # ---- whole-tensor DMA xbar transposes to d-major ----
                 hT = tr_pool.tile([D, E, CHUNK], fp16, name="hT")
                 if h_pe_tr:
                     for gi in range(E // EG):
                         htp = psk_pool.tile([D, EG, CHUNK], fp16, name="htp",
                                             tag="ktp")
                         for j in range(EG):
                             nc.tensor.transpose(
                                 htp[:, j], h16[:, gi * EG + j], ident16[:]
                             )
                         if h_evac_act:
                             nc.scalar.copy(hT[:, gi * EG : (gi + 1) * EG], htp[:])
                         else:
                             nc.vector.tensor_copy(hT[:, gi * EG : (gi + 1) * EG], htp[:])
                 else:
                     nc.sync.dma_start_transpose(out=hT[:], in_=h16)
                 kT = tr_pool.tile([D, E, CHUNK], fp16, name="kT")
                 if k_pe_tr:
                     for gi in range(E // EG):
                         ktp = psk_pool.tile([D, EG, CHUNK], fp16, name="ktp")
                         for j in range(EG):
                             nc.tensor.transpose(
                                 ktp[:, j], k16[:, gi * EG + j], ident16[:]
                             )
                         nc.scalar.copy(kT[:, gi * EG : (gi + 1) * EG], ktp[:])
                 else:
                     nc.sync.dma_start_transpose(out=kT[:], in_=k16)
                 sT = tr_pool.tile([D, CHUNK], fp16, name="sT")
                 nc.sync.dma_start(out=sT[:], in_=s16, transpose=True)

                 if ablate == 'xpose':
                     nc.sync.dma_start(
                         out=out_v[c][:, : E * D // 2],
                         in_=hT[:].rearrange("p e d -> p (e d)").bitcast(fp32),
                     )
                     nc.sync.dma_start(
                         out=out_v[c][:, E * D // 2 :],
                         in_=kT[:].rearrange("p e d -> p (e d)").bitcast(fp32),
                     )
                     continue
                 # ---- gates input: t2T = (hT + kT) * sT  (GpSimd) ----
                 hkT = tr_pool.tile([D, E, CHUNK], fp16, name="hkT")
                 nc.gpsimd.tensor_tensor(hkT[:], hT[:], kT[:], OP.add)
                 sTb = sT[:].unsqueeze(1).broadcast_to([D, E, CHUNK])
                 t2T = hkT  # in-place: hkT is dead after this multiply
                 nc.gpsimd.tensor_tensor(t2T[:], hkT[:], sTb, OP.mult)

                 # ---- gates reduce over d on PE; sigmoid on ScalarE ----
                 gps = psg_pool.tile([CHUNK, E], fp32, name="gps")
                 for e in range(E):
                     nc.tensor.matmul(
                         gps[:, e : e + 1], t2T[:, e], ones16[:],
                         start=True, stop=True,
                     )
                 g32 = sm_pool.tile([CHUNK, E], fp32, name="g32")
                 nc.scalar.activation(g32[:], gps[:], AF.Sigmoid)

                 # ---- main matmuls + tanh ----
                 ht16 = bf_pool.tile([CHUNK, E, D], fp16, name="ht16")
                 for gi in range(E // EG):
                     ps = psm_pool.tile([CHUNK, EG, D], fp32, name="ps")
                     for j in range(EG):
                         e = gi * EG + j
                         nc.tensor.matmul(
                             ps[:, j], hT[:, e], u16c[:], start=True, stop=False
                         )
                         nc.tensor.matmul(
                             ps[:, j], kT[:, e], v16c[:], start=False, stop=False
                         )
                         nc.tensor.matmul(
                             ps[:, j], sT[:], w16c[:], start=False, stop=True
                         )
                     nc.scalar.activation(
                         ht16[:, gi * EG : (gi + 1) * EG], ps[:], AF.Tanh
                     )

                 if ablate == 'compute':
                     nc.sync.dma_start(
                         out=out_v[c][:, : E * D // 2],
                         in_=ht16[:].rearrange("p e d -> p (e d)").bitcast(fp32),
                     )
                     nc.sync.dma_start(
                         out=out_v[c][:, E * D // 2 :],
                         in_=t2T[:].rearrange("p e d -> p (e d)").bitcast(fp32),
                     )
                     continue
                 # ---- update u = g * h_tilda + h (VectorE, fused, in place
                 # over ht16: the tanh output is dead after this) ----
                 u16 = ht16
                 for e in range(E):
                     nc.vector.scalar_tensor_tensor(
                         u16[:, e], ht16[:, e], g32[:, e : e + 1], h16[:, e],
                         OP.mult, OP.add,
                     )

                 # ---- sum(u^2) via bn_stats (6 outputs/partition per call) ----
                 bn = sm_pool.tile([CHUNK, E, 6], fp32, name="bn")
                 for e in range(E):
                     nc.vector.bn_stats(bn[:, e, :], u16[:, e])
                 # normsq = 64*(mu_even^2 + mu_odd^2) + (cvar_even + cvar_odd)
                 t_a = sm_pool.tile([CHUNK, E], fp32, name="t_a")
                 nc.vector.tensor_tensor(t_a[:], bn[:, :, 1], bn[:, :, 1], OP.mult)
                 t_b = sm_pool.tile([CHUNK, E], fp32, name="t_b")
                 nc.vector.tensor_tensor(t_b[:], bn[:, :, 4], bn[:, :, 4], OP.mult)
                 t_ab = sm_pool.tile([CHUNK, E], fp32, name="t_ab")
                 nc.vector.tensor_tensor(t_ab[:], t_a[:], t_b[:], OP.add)
                 t_c = sm_pool.tile([CHUNK, E], fp32, name="t_c")
                 nc.vector.tensor_tensor(t_c[:], bn[:, :, 2], bn[:, :, 5], OP.add)
                 a32 = sm_pool.tile([CHUNK, E], fp32, name="a32")
                 nc.vector.scalar_tensor_tensor(
                     a32[:], t_ab[:], 64.0, t_c[:], OP.mult, OP.add
                 )
                 nc.vector.tensor_scalar(a32[:], a32[:], 1e-12, None, op0=OP.max)

                 # ---- r = rsqrt(a): bit-trick seed + Newton iterations ----
                 ti = sm_pool.tile([CHUNK, E], int32, name="ti")
                 nc.vector.tensor_scalar(
                     ti[:], a32[:].bitcast(int32), 1, None,
                     op0=OP.logical_shift_right,
                 )
                 yi = sm_pool.tile([CHUNK, E], int32, name="yi")
                 nc.vector.tensor_tensor(yi[:], magic[:], ti[:], OP.subtract)
                 y = yi[:].bitcast(fp32)
                 for _ in range(newton_iters):
                     y2 = sm_pool.tile([CHUNK, E], fp32, name="y2")
                     nc.vector.tensor_tensor(y2[:], y, y, OP.mult)
                     tt = sm_pool.tile([CHUNK, E], fp32, name="tt")
                     nc.vector.tensor_tensor(tt[:], a32[:], y2[:], OP.mult)
                     ww = sm_pool.tile([CHUNK, E], fp32, name="ww")
                     nc.vector.tensor_scalar(
                         ww[:], tt[:], -0.5, 1.5, op0=OP.mult, op1=OP.add
                     )
                     yn = sm_pool.tile([CHUNK, E], fp32, name="yn")
                     nc.vector.tensor_tensor(yn[:], y, ww[:], OP.mult)
                     y = yn[:]

                 # ---- scale out = u * r and store ----
                 if store16:
                     o16 = bf_pool.tile([CHUNK, E, D], fp16, name="o16")
                     for e in range(E):
                         nc.vector.scalar_tensor_tensor(
                             o16[:, e], u16[:, e], y[:, e : e + 1], u16[:, e],
                             OP.mult, OP.bypass,
                         )
                     nc.gpsimd.dma_start(
                         out=out_v[c], in_=o16[:].rearrange("p e d -> p (e d)")
                     )
                 else:
                     o32 = io_pool.tile([CHUNK, E, D], fp32, name="o32")
                     for e in range(E):
                         if e < scale_on_act:
                             nc.scalar.mul(o32[:, e], u16[:, e], y[:, e : e + 1])
                         elif scale_ts:
                             nc.vector.tensor_scalar(
                                 o32[:, e], u16[:, e], y[:, e : e + 1], None,
                                 op0=OP.mult,
                             )
                         else:
                             nc.vector.scalar_tensor_tensor(
                                 o32[:, e], u16[:, e], y[:, e : e + 1], u16[:, e],
                                 OP.mult, OP.bypass,
                             )
                     nc.sync.dma_start(
                         out=out_v[c], in_=o32[:].rearrange("p e d -> p (e d)")
                     )

    nc.compile()
    return nc


def _get_nc():
    if "nc" not in _CACHE:
        _CACHE["nc"] = _build_nc()
    return _CACHE["nc"]


def kernel(encoded_sents, prev_states, keys, U, V, W):
    import sys

    if "/opt/trn_rl_repo" not in sys.path:
        sys.path.insert(0, "/opt/trn_rl_repo")
    from concourse.bass_utils import run_bass_kernel_spmd

    nc = _get_nc()
    enc = np.ascontiguousarray(np.asarray(encoded_sents, dtype=np.float32))
    prev = np.ascontiguousarray(np.asarray(prev_states, dtype=np.float32))
    kys = np.ascontiguousarray(np.asarray(keys, dtype=np.float32))
    U = np.ascontiguousarray(np.asarray(U, dtype=np.float32))
    V = np.ascontiguousarray(np.asarray(V, dtype=np.float32))
    W = np.ascontiguousarray(np.asarray(W, dtype=np.float32))

    in_maps = []
    for i in range(N_CORES):
        lo, hi = i * B_LOC, (i + 1) * B_LOC
        in_maps.append(
            {
                "enc": enc[lo:hi],
                "prev": prev[lo:hi],
                "keys": kys[lo:hi],
                "U": U,
                "V": V,
                "W": W,
            }
        )

    res = run_bass_kernel_spmd(nc, in_maps, list(range(N_CORES)))
    out = np.concatenate([res.results[i]["out"] for i in range(N_CORES)], axis=0)
    return out.astype(np.float32)



# revision 4
# speedup vs baseline: 1.6173x; 1.6173x over previous
"""Trainium2 Bass kernel for nn_EntityCell (scatter_memory).

Math (per batch row b, entity e):
    gates = sigmoid(sum_d(s * (h + k)))              [B, E]
    h_tilda = tanh(h @ U + k @ V + (s @ W)[:, None]) [B, E, D]
    updated = h + gates[:, :, None] * h_tilda
    out = updated / sqrt(max(sum_d(updated^2), 1e-12))

Sharding: pure data parallel over the batch dim across 8 NeuronCores.

Numerics: inputs are cast to fp16 on the host (rel err ~5e-4, tolerance is
2e-2) and the device stores an fp16 output that the host upcasts to fp32.
This halves HBM traffic on both sides; all on-chip matmul is fp16 with fp32
PSUM accumulation.

Per-core dataflow (B_loc=1024 rows, 8 chunks of 128 rows):
  - DMA loads fp16 row-major chunks (5 KB contiguous per partition line).
  - Transposes to d-major: h via one DMA xbar transpose, k (and s) on PE
    via identity matmuls, evacuated from PSUM by ScalarE.
  - Gates: one DVE multiply (hT|kT stacked) by broadcast sT, reduced over
    d on PE by ones-matmuls, sigmoid on ScalarE.
  - Mains: per-entity fp16 matmuls hT_e@U + kT_e@V + sT@W accumulated in
    PSUM; tanh evac to fp16 on ScalarE.
  - Epilogue on DVE with batched ops: per-e tensor_scalar g-mult (4x mode),
    one big add for h, one tensor_tensor square, one segmented
    tensor_reduce for sum(u^2), 1-iteration Newton rsqrt from the bit-trick
    seed, per-e tensor_scalar final scale to fp16.
"""

import numpy as np
from contextlib import nullcontext as _nullctx

B, E, D = 8192, 20, 128
N_CORES = 8
B_LOC = B // N_CORES
CHUNK = 128
N_CHUNKS = B_LOC // CHUNK

_CACHE = {}


def _build_nc(reps=1, loop_n=None, ablate=None, h_tr='dma', gate_mode='fused',
              sq_eng='dve', upd_mode='ts', eg=4, eg_t=4, newton_iters=1,
              kevac='act', hevac='dve', io_bufs=3, tr_bufs=2, bf_bufs=2,
              psm_bufs=4, psk_bufs=2, psg_bufs=1, scale_eng='dve'):
    import concourse.tile as tile
    from concourse import bacc, mybir
    from concourse.masks import make_identity
    from contextlib import ExitStack

    fp32 = mybir.dt.float32
    fp16 = mybir.dt.float16
    int32 = mybir.dt.int32
    AF = mybir.ActivationFunctionType
    OP = mybir.AluOpType

    nc = bacc.Bacc("TRN2", target_bir_lowering=False, debug=False)
    enc_d = nc.declare_dram_parameter("enc", [B_LOC, D], fp16, isOutput=False)
    prev_d = nc.declare_dram_parameter("prev", [B_LOC, E, D], fp16, isOutput=False)
    keys_d = nc.declare_dram_parameter("keys", [B_LOC, E, D], fp16, isOutput=False)
    u_d = nc.declare_dram_parameter("U", [D, D], fp16, isOutput=False)
    v_d = nc.declare_dram_parameter("V", [D, D], fp16, isOutput=False)
    w_d = nc.declare_dram_parameter("W", [D, D], fp16, isOutput=False)
    out_d = nc.declare_dram_parameter("out", [B_LOC, E, D], fp16, isOutput=True)

    prev_v = prev_d[:].rearrange("(n p) e d -> n p (e d)", p=CHUNK)
    keys_v = keys_d[:].rearrange("(n p) e d -> n p (e d)", p=CHUNK)
    enc_v = enc_d[:].rearrange("(n p) d -> n p d", p=CHUNK)
    out_v = out_d[:].rearrange("(n p) e d -> n p (e d)", p=CHUNK)

    NG = E // eg       # main matmul groups per chunk
    NT = E // eg_t     # transpose groups per chunk

    with ExitStack() as ctx:
        tc = ctx.enter_context(tile.TileContext(nc))
        const_pool = ctx.enter_context(tc.tile_pool(name="const", bufs=1))
        io_pool = ctx.enter_context(tc.tile_pool(name="io", bufs=io_bufs))
        tr_pool = ctx.enter_context(tc.tile_pool(name="tr", bufs=tr_bufs))
        bf_pool = ctx.enter_context(tc.tile_pool(name="bf", bufs=bf_bufs))
        sm_pool = ctx.enter_context(tc.tile_pool(name="sm", bufs=4))
        psm_pool = ctx.enter_context(tc.tile_pool(name="psm", bufs=psm_bufs, space="PSUM"))
        psk_pool = ctx.enter_context(tc.tile_pool(name="psk", bufs=psk_bufs, space="PSUM"))
        pss_pool = ctx.enter_context(tc.tile_pool(name="pss", bufs=1, space="PSUM"))
        psg_pool = (
            ctx.enter_context(tc.tile_pool(name="psg", bufs=psg_bufs, space="PSUM"))
            if gate_mode != 'row' else None
        )

        # ---- constants ----
        u16c = const_pool.tile([D, D], fp16)
        v16c = const_pool.tile([D, D], fp16)
        w16c = const_pool.tile([D, D], fp16)
        nc.sync.dma_start(u16c[:], u_d[:])
        nc.sync.dma_start(v16c[:], v_d[:])
        nc.sync.dma_start(w16c[:], w_d[:])
        ones16 = const_pool.tile([D, 1], fp16)
        nc.gpsimd.memset(ones16[:], 1.0)
        magic = const_pool.tile([CHUNK, E], int32)
        nc.gpsimd.memset(magic[:], 0x5F3759DF)
        ident16 = const_pool.tile([D, D], fp16)
        make_identity(nc, ident16[:])

        loop_cm = (
            tc.For_i(0, loop_n, 1, hint_engines=tuple(mybir.ALL_ENGINES))
            if loop_n is not None
            else _nullctx()
        )
        with loop_cm:
         for cp in range(N_CHUNKS * reps):
            n = cp % N_CHUNKS
            # ---- loads ----
            h16 = io_pool.tile([CHUNK, E, D], fp16, name="h16")
            nc.sync.dma_start(h16[:].rearrange("p e d -> p (e d)"), prev_v[n])
            k16 = io_pool.tile([CHUNK, E, D], fp16, name="k16")
            nc.sync.dma_start(k16[:].rearrange("p e d -> p (e d)"), keys_v[n])
            s16 = io_pool.tile([CHUNK, D], fp16, name="s16")
            nc.sync.dma_start(s16[:], enc_v[n])

            if ablate == 'dma':
                nc.sync.dma_start(
                    out=out_v[n], in_=h16[:].rearrange("p e d -> p (e d)")
                )
                continue

            # ---- transposes to d-major ----
            # sT on PE
            stp = pss_pool.tile([D, CHUNK], fp16, name="stp")
            nc.tensor.transpose(stp[:], s16[:], ident16[:])
            sT = tr_pool.tile([D, CHUNK], fp16, name="sT")
            nc.scalar.copy(sT[:], stp[:])

            # hT and kT stacked in one tile so the gate multiply is one op
            hkT = tr_pool.tile([D, 2, E, CHUNK], fp16, name="hkT")
            if h_tr == 'dma':
                nc.sync.dma_start_transpose(out=hkT[:, 0], in_=h16[:])
            else:
                for gi in range(NT):
                    htp = psk_pool.tile([D, eg_t, CHUNK], fp16, name="ktp",
                                        tag="ktp")
                    for j in range(eg_t):
                        nc.tensor.transpose(
                            htp[:, j], h16[:, gi * eg_t + j], ident16[:]
                        )
                    if hevac == 'act':
                        nc.scalar.copy(hkT[:, 0, gi * eg_t:(gi + 1) * eg_t], htp[:])
                    else:
                        nc.vector.tensor_copy(hkT[:, 0, gi * eg_t:(gi + 1) * eg_t], htp[:])
            for gi in range(NT):
                ktp = psk_pool.tile([D, eg_t, CHUNK], fp16, name="ktp", tag="ktp")
                for j in range(eg_t):
                    nc.tensor.transpose(
                        ktp[:, j], k16[:, gi * eg_t + j], ident16[:]
                    )
                if kevac == 'act':
                    nc.scalar.copy(hkT[:, 1, gi * eg_t:(gi + 1) * eg_t], ktp[:])
                else:
                    nc.vector.tensor_copy(hkT[:, 1, gi * eg_t:(gi + 1) * eg_t], ktp[:])

            if ablate == 'xpose':
                nc.sync.dma_start(
                    out=out_v[n],
                    in_=hkT[:, 0].rearrange("p e d -> p (e d)"),
                )
                continue

            # ---- gates ----
            g32 = sm_pool.tile([CHUNK, E], fp32, name="g32")
            if gate_mode == 'row':
                # row-major: t2 = (h+k)*s_b, segmented reduce over d
                hk = bf_pool.tile([CHUNK, E, D], fp16, name="hk")
                nc.vector.tensor_tensor(hk[:], h16[:], k16[:], OP.add)
                s16b = s16[:].unsqueeze(1).broadcast_to([CHUNK, E, D])
                nc.vector.tensor_tensor(hk[:], hk[:], s16b, OP.mult)
                gss = sm_pool.tile([CHUNK, E], fp32, name="gss")
                nc.vector.tensor_reduce(
                    gss[:], hk[:], axis=mybir.AxisListType.X, op=OP.add
                )
                nc.scalar.activation(g32[:], gss[:], AF.Sigmoid)
            else:
                t2T = tr_pool.tile([D, 2, E, CHUNK], fp16, name="t2T")
                sTb = sT[:].unsqueeze(1).broadcast_to([D, 2 * E, CHUNK])
                if gate_mode == 'gps':
                    nc.vector.tensor_tensor(
                        t2T[:, 0], hkT[:, 0],
                        sT[:].unsqueeze(1).broadcast_to([D, E, CHUNK]), OP.mult
                    )
                    nc.gpsimd.tensor_tensor(
                        t2T[:, 1], hkT[:, 1],
                        sT[:].unsqueeze(1).broadcast_to([D, E, CHUNK]), OP.mult
                    )
                else:  # fused
                    nc.vector.tensor_tensor(
                        t2T[:].rearrange("d a e c -> d (a e) c"),
                        hkT[:].rearrange("d a e c -> d (a e) c"),
                        sTb, OP.mult,
                    )
                gps = psg_pool.tile([CHUNK, E], fp32, name="gps")
                for e in range(E):
                    nc.tensor.matmul(
                        gps[:, e:e + 1], t2T[:, 0, e], ones16[:],
                        start=True, stop=False,
                    )
                    nc.tensor.matmul(
                        gps[:, e:e + 1], t2T[:, 1, e], ones16[:],
                        start=False, stop=True,
                    )
                nc.scalar.activation(g32[:], gps[:], AF.Sigmoid)

            # ---- main matmuls + tanh ----
            ht16 = bf_pool.tile([CHUNK, E, D], fp16, name="ht16")
            for gi in range(NG):
                ps = psm_pool.tile([CHUNK, eg, D], fp32, name="ps")
                for j in range(eg):
                    e = gi * eg + j
                    nc.tensor.matmul(
                        ps[:, j], hkT[:, 0, e], u16c[:], start=True, stop=False
                    )
                    nc.tensor.matmul(
                        ps[:, j], hkT[:, 1, e], v16c[:], start=False, stop=False
                    )
                    nc.tensor.matmul(
                        ps[:, j], sT[:], w16c[:], start=False, stop=True
                    )
                nc.scalar.activation(
                    ht16[:, gi * eg:(gi + 1) * eg], ps[:], AF.Tanh
                )

            if ablate == 'compute':
                nc.sync.dma_start(
                    out=out_v[n], in_=ht16[:].rearrange("p e d -> p (e d)")
                )
                continue

            # ---- update u = g*t + h ----
            u16 = bf_pool.tile([CHUNK, E, D], fp16, name="u16")
            if upd_mode == 'stt':
                for e in range(E):
                    nc.vector.scalar_tensor_tensor(
                        u16[:, e], ht16[:, e], g32[:, e:e + 1], h16[:, e],
                        OP.mult, OP.add,
                    )
            else:  # 'ts': per-e 4x-mode TS then one big add
                for e in range(E):
                    nc.vector.tensor_scalar(
                        u16[:, e], ht16[:, e], g32[:, e:e + 1], None,
                        op0=OP.mult,
                    )
                nc.vector.tensor_tensor(u16[:], u16[:], h16[:], OP.add)

            # ---- sum(u^2) ----
            u2 = bf_pool.tile([CHUNK, E, D], fp16, name="u2")
            if sq_eng == 'act':
                nc.scalar.activation(u2[:], u16[:], AF.Square)
            elif sq_eng == 'gps':
                nc.gpsimd.tensor_tensor(u2[:], u16[:], u16[:], OP.mult)
            else:
                nc.vector.tensor_tensor(u2[:], u16[:], u16[:], OP.mult)
            ss = sm_pool.tile([CHUNK, E], fp32, name="ss")
            nc.vector.tensor_reduce(
                ss[:], u2[:], axis=mybir.AxisListType.X, op=OP.add
            )

            # ---- r = rsqrt(ss): bit-trick seed + Newton ----
            ti = sm_pool.tile([CHUNK, E], int32, name="ti")
            nc.vector.tensor_scalar(
                ti[:], ss[:].bitcast(int32), 1, None,
                op0=OP.logical_shift_right,
            )
            yi = sm_pool.tile([CHUNK, E], int32, name="yi")
            nc.vector.tensor_tensor(yi[:], magic[:], ti[:], OP.subtract)
            y = yi[:].bitcast(fp32)
            for _ in range(newton_iters):
                y2 = sm_pool.tile([CHUNK, E], fp32, name="y2")
                nc.vector.tensor_tensor(y2[:], y, y, OP.mult)
                tt = sm_pool.tile([CHUNK, E], fp32, name="tt")
                nc.vector.tensor_tensor(tt[:], ss[:], y2[:], OP.mult)
                ww = sm_pool.tile([CHUNK, E], fp32, name="ww")
                nc.vector.tensor_scalar(
                    ww[:], tt[:], -0.5, 1.5, op0=OP.mult, op1=OP.add
                )
                yn = sm_pool.tile([CHUNK, E], fp32, name="yn")
                nc.vector.tensor_tensor(yn[:], y, ww[:], OP.mult)
                y = yn[:]

            # ---- scale and store fp16 ----
            o16 = bf_pool.tile([CHUNK, E, D], fp16, name="o16")
            for e in range(E):
                if scale_eng == 'act':
                    nc.scalar.mul(o16[:, e], u16[:, e], y[:, e:e + 1])
                else:
                    nc.vector.tensor_scalar(
                        o16[:, e], u16[:, e], y[:, e:e + 1], None, op0=OP.mult
                    )
            nc.sync.dma_start(
                out=out_v[n], in_=o16[:].rearrange("p e d -> p (e d)")
            )

    nc.compile()
    return nc


def _get_nc():
    if "nc" not in _CACHE:
        _CACHE["nc"] = _build_nc()
    return _CACHE["nc"]


def make_in_maps(encoded_sents, prev_states, keys, U, V, W):
    enc = np.ascontiguousarray(np.asarray(encoded_sents, dtype=np.float16))
    prev = np.ascontiguousarray(np.asarray(prev_states, dtype=np.float16))
    kys = np.ascontiguousarray(np.asarray(keys, dtype=np.float16))
    U = np.ascontiguousarray(np.asarray(U, dtype=np.float16))
    V = np.ascontiguousarray(np.asarray(V, dtype=np.float16))
    W = np.ascontiguousarray(np.asarray(W, dtype=np.float16))
    in_maps = []
    for i in range(N_CORES):
        lo, hi = i * B_LOC, (i + 1) * B_LOC
        in_maps.append(
            {
                "enc": enc[lo:hi],
                "prev": prev[lo:hi],
                "keys": kys[lo:hi],
                "U": U,
                "V": V,
                "W": W,
            }
        )
    return in_maps


def kernel(encoded_sents, prev_states, keys, U, V, W):
    import sys

    if "/opt/trn_rl_repo" not in sys.path:
        sys.path.insert(0, "/opt/trn_rl_repo")
    from concourse.bass_utils import run_bass_kernel_spmd

    nc = _get_nc()
    in_maps = make_in_maps(encoded_sents, prev_states, keys, U, V, W)
    res = run_bass_kernel_spmd(nc, in_maps, list(range(N_CORES)))
    out = np.concatenate([res.results[i]["out"] for i in range(N_CORES)], axis=0)
    return out.astype(np.float32)


# revision 14
# speedup vs baseline: 2.5839x; 1.5977x over previous
"""Trainium2 Bass kernel for nn_EntityCell (scatter_memory).

Math (per batch row b, entity e):
    gates = sigmoid(sum_d(s * (h + k)))              [B, E]
    h_tilda = tanh(h @ U + k @ V + (s @ W)[:, None]) [B, E, D]
    updated = h + gates[:, :, None] * h_tilda
    out = updated / sqrt(max(sum_d(updated^2), 1e-12))

Sharding: pure data parallel over the batch dim across 8 NeuronCores.

Host-side layout prep (part of the sharding step in kernel()):
  - inputs cast to fp16 (rel err ~5e-4 vs the 2e-2 tolerance); output is
    stored fp16 on device and upcast to fp32 on the host.
  - prev/keys/enc are ALSO pre-transposed per 128-row chunk to d-major
    [chunk, D, E, rows] so the device needs no on-chip transposes at all
    (PE matmul contracts over the partition dim, which must be d).
    prev is additionally kept row-major for the update step: HBM cost of
    loading h twice is ~1.8us/chunk against >6us/chunk of engine time for
    on-chip transposition + PSUM evacuation.

Per-core dataflow (B_loc=1024 rows, 8 chunks of 128 rows):
  - DMA: 5 transfers/chunk (hT, kT into one stacked tile; sT; h row-major;
    store), every partition line >= 5KB contiguous.
  - DVE: one fused multiply t2T = [hT|kT] * broadcast(sT) feeds the gate
    reduction; per-e tensor_scalar g-mult (4x mode), one big h-add, the
    segmented sum(u^2) reduce, 1-step Newton rsqrt, per-e final scale.
  - PE: per-e ones-matmul gate reduction over d; per-e fp16 matmuls
    hT_e@U + kT_e@V + sT@W accumulated in fp32 PSUM.
  - Act: sigmoid, tanh evac from PSUM to fp16, and the u^2 square (keeps
    front-stage Act FIFO free of late-stage dependencies).
"""

import numpy as np
from contextlib import nullcontext as _nullctx

B, E, D = 8192, 20, 128
N_CORES = 8
B_LOC = B // N_CORES
CHUNK = 128
N_CHUNKS = B_LOC // CHUNK

_CACHE = {}


def _groups(total, size):
    out = []
    lo = 0
    while lo < total:
        out.append((lo, min(size, total - lo)))
        lo += min(size, total - lo)
    return out


def _build_nc(reps=1, loop_n=None, ablate=None, gate_mode='fused',
              sq_eng='act', upd_mode='ts', eg=4, newton_iters=1,
              io_bufs=4, tr_bufs=3, bf_bufs=3, psm_bufs=4, psg_bufs=1,
              scale_eng='dve', add_eng='dve'):
    import concourse.tile as tile
    from concourse import bacc, mybir
    from contextlib import ExitStack

    fp32 = mybir.dt.float32
    fp16 = mybir.dt.float16
    int32 = mybir.dt.int32
    AF = mybir.ActivationFunctionType
    OP = mybir.AluOpType

    nc = bacc.Bacc("TRN2", target_bir_lowering=False, debug=False)
    prevT_d = nc.declare_dram_parameter("prevT", [N_CHUNKS, D, E, CHUNK], fp16, isOutput=False)
    keysT_d = nc.declare_dram_parameter("keysT", [N_CHUNKS, D, E, CHUNK], fp16, isOutput=False)
    encT_d = nc.declare_dram_parameter("encT", [N_CHUNKS, D, CHUNK], fp16, isOutput=False)
    prev_d = nc.declare_dram_parameter("prev", [B_LOC, E, D], fp16, isOutput=False)
    u_d = nc.declare_dram_parameter("U", [D, D], fp16, isOutput=False)
    v_d = nc.declare_dram_parameter("V", [D, D], fp16, isOutput=False)
    w_d = nc.declare_dram_parameter("W", [D, D], fp16, isOutput=False)
    out_d = nc.declare_dram_parameter("out", [B_LOC, E, D], fp16, isOutput=True)

    prev_v = prev_d[:].rearrange("(n p) e d -> n p (e d)", p=CHUNK)
    out_v = out_d[:].rearrange("(n p) e d -> n p (e d)", p=CHUNK)

    G_MAIN = _groups(E, eg)

    with ExitStack() as ctx:
        tc = ctx.enter_context(tile.TileContext(nc))
        const_pool = ctx.enter_context(tc.tile_pool(name="const", bufs=1))
        io_pool = ctx.enter_context(tc.tile_pool(name="io", bufs=io_bufs))
        tr_pool = ctx.enter_context(tc.tile_pool(name="tr", bufs=tr_bufs))
        bf_pool = ctx.enter_context(tc.tile_pool(name="bf", bufs=bf_bufs))
        sm_pool = ctx.enter_context(tc.tile_pool(name="sm", bufs=6))
        psm_pool = ctx.enter_context(tc.tile_pool(name="psm", bufs=psm_bufs, space="PSUM"))
        psg_pool = ctx.enter_context(tc.tile_pool(name="psg", bufs=psg_bufs, space="PSUM"))

        # ---- constants ----
        u16c = const_pool.tile([D, D], fp16)
        v16c = const_pool.tile([D, D], fp16)
        w16c = const_pool.tile([D, D], fp16)
        nc.sync.dma_start(u16c[:], u_d[:])
        nc.sync.dma_start(v16c[:], v_d[:])
        nc.sync.dma_start(w16c[:], w_d[:])
        ones16 = const_pool.tile([D, 1], fp16)
        nc.gpsimd.memset(ones16[:], 1.0)
        magic = const_pool.tile([CHUNK, E], int32)
        nc.gpsimd.memset(magic[:], 0x5F3759DF)

        loop_cm = (
            tc.For_i(0, loop_n, 1, hint_engines=tuple(mybir.ALL_ENGINES))
            if loop_n is not None
            else _nullctx()
        )
        with loop_cm:
         for cp in range(N_CHUNKS * reps):
            n = cp % N_CHUNKS
            # ---- loads (hT/kT pre-transposed on host) ----
            hkT = tr_pool.tile([D, 2, E, CHUNK], fp16, name="hkT")
            nc.sync.dma_start(
                hkT[:, 0].rearrange("p e c -> p (e c)"),
                prevT_d[n].rearrange("p e c -> p (e c)"),
            )
            nc.sync.dma_start(
                hkT[:, 1].rearrange("p e c -> p (e c)"),
                keysT_d[n].rearrange("p e c -> p (e c)"),
            )
            sT = tr_pool.tile([D, CHUNK], fp16, name="sT")
            nc.sync.dma_start(sT[:], encT_d[n])
            h16 = io_pool.tile([CHUNK, E, D], fp16, name="h16")
            nc.sync.dma_start(h16[:].rearrange("p e d -> p (e d)"), prev_v[n])

            if ablate == 'dma':
                nc.sync.dma_start(
                    out=out_v[n], in_=h16[:].rearrange("p e d -> p (e d)")
                )
                continue

            # ---- gates ----
            g32 = sm_pool.tile([CHUNK, E], fp32, name="g32")
            t2T = tr_pool.tile([D, 2, E, CHUNK], fp16, name="t2T")
            sTb = sT[:].unsqueeze(1).broadcast_to([D, 2 * E, CHUNK])
            nc.vector.tensor_tensor(
                t2T[:].rearrange("d a e c -> d (a e) c"),
                hkT[:].rearrange("d a e c -> d (a e) c"),
                sTb, OP.mult,
            )
            gps = psg_pool.tile([CHUNK, E], fp32, name="gps")
            for e in range(E):
                nc.tensor.matmul(
                    gps[:, e:e + 1], t2T[:, 0, e], ones16[:],
                    start=True, stop=False,
                )
                nc.tensor.matmul(
                    gps[:, e:e + 1], t2T[:, 1, e], ones16[:],
                    start=False, stop=True,
                )
            nc.scalar.activation(g32[:], gps[:], AF.Sigmoid)

            if ablate == 'gates':
                nc.sync.dma_start(
                    out=out_v[n],
                    in_=t2T[:, 0].rearrange("p e d -> p (e d)"),
                )
                continue

            # ---- main matmuls + tanh ----
            ht16 = bf_pool.tile([CHUNK, E, D], fp16, name="ht16")
            for lo, sz in G_MAIN:
                ps = psm_pool.tile([CHUNK, eg, D], fp32, name="ps")
                for j in range(sz):
                    e = lo + j
                    nc.tensor.matmul(
                        ps[:, j], hkT[:, 0, e], u16c[:], start=True, stop=False
                    )
                    nc.tensor.matmul(
                        ps[:, j], hkT[:, 1, e], v16c[:], start=False, stop=False
                    )
                    nc.tensor.matmul(
                        ps[:, j], sT[:], w16c[:], start=False, stop=True
                    )
                nc.scalar.activation(
                    ht16[:, lo:lo + sz], ps[:, :sz], AF.Tanh
                )

            if ablate == 'compute':
                nc.sync.dma_start(
                    out=out_v[n], in_=ht16[:].rearrange("p e d -> p (e d)")
                )
                continue

            # ---- update u = g*t + h ----
            u16 = bf_pool.tile([CHUNK, E, D], fp16, name="u16")
            if upd_mode == 'stt':
                for e in range(E):
                    nc.vector.scalar_tensor_tensor(
                        u16[:, e], ht16[:, e], g32[:, e:e + 1], h16[:, e],
                        OP.mult, OP.add,
                    )
            else:  # 'ts': per-e 4x-mode TS then one big add
                for e in range(E):
                    nc.vector.tensor_scalar(
                        u16[:, e], ht16[:, e], g32[:, e:e + 1], None,
                        op0=OP.mult,
                    )
                if add_eng == 'gps':
                    nc.gpsimd.tensor_tensor(u16[:], u16[:], h16[:], OP.add)
                else:
                    nc.vector.tensor_tensor(u16[:], u16[:], h16[:], OP.add)

            if ablate == 'update':
                nc.sync.dma_start(
                    out=out_v[n], in_=u16[:].rearrange("p e d -> p (e d)")
                )
                continue

            # ---- sum(u^2) ----
            u2 = bf_pool.tile([CHUNK, E, D], fp16, name="u2")
            if sq_eng == 'act':
                nc.scalar.activation(u2[:], u16[:], AF.Square)
            elif sq_eng == 'gps':
                nc.gpsimd.tensor_tensor(u2[:], u16[:], u16[:], OP.mult)
            else:
                nc.vector.tensor_tensor(u2[:], u16[:], u16[:], OP.mult)
            ss = sm_pool.tile([CHUNK, E], fp32, name="ss")
            nc.vector.tensor_reduce(
                ss[:], u2[:], axis=mybir.AxisListType.X, op=OP.add
            )

            # ---- r = rsqrt(ss): bit-trick seed + Newton ----
            ti = sm_pool.tile([CHUNK, E], int32, name="ti")
            nc.vector.tensor_scalar(
                ti[:], ss[:].bitcast(int32), 1, None,
                op0=OP.logical_shift_right,
            )
            yi = sm_pool.tile([CHUNK, E], int32, name="yi")
            nc.vector.tensor_tensor(yi[:], magic[:], ti[:], OP.subtract)
            y = yi[:].bitcast(fp32)
            for _ in range(newton_iters):
                y2 = sm_pool.tile([CHUNK, E], fp32, name="y2")
                nc.vector.tensor_tensor(y2[:], y, y, OP.mult)
                tt = sm_pool.tile([CHUNK, E], fp32, name="tt")
                nc.vector.tensor_tensor(tt[:], ss[:], y2[:], OP.mult)
                ww = sm_pool.tile([CHUNK, E], fp32, name="ww")
                nc.vector.tensor_scalar(
                    ww[:], tt[:], -0.5, 1.5, op0=OP.mult, op1=OP.add
                )
                yn = sm_pool.tile([CHUNK, E], fp32, name="yn")
                nc.vector.tensor_tensor(yn[:], y, ww[:], OP.mult)
                y = yn[:]

            if ablate == 'norm':
                nc.sync.dma_start(
                    out=out_v[n], in_=u16[:].rearrange("p e d -> p (e d)")
                )
                continue

            # ---- scale and store fp16 ----
            o16 = bf_pool.tile([CHUNK, E, D], fp16, name="o16")
            for e in range(E):
                if scale_eng == 'act':
                    nc.scalar.mul(o16[:, e], u16[:, e], y[:, e:e + 1])
                else:
                    nc.vector.tensor_scalar(
                        o16[:, e], u16[:, e], y[:, e:e + 1], None, op0=OP.mult
                    )
            nc.sync.dma_start(
                out=out_v[n], in_=o16[:].rearrange("p e d -> p (e d)")
            )

    nc.compile()
    return nc


def _get_nc():
    if "nc" not in _CACHE:
        _CACHE["nc"] = _build_nc()
    return _CACHE["nc"]


def make_in_maps(encoded_sents, prev_states, keys, U, V, W):
    enc = np.asarray(encoded_sents, dtype=np.float16)
    prev = np.asarray(prev_states, dtype=np.float16)
    kys = np.asarray(keys, dtype=np.float16)
    U = np.ascontiguousarray(np.asarray(U, dtype=np.float16))
    V = np.ascontiguousarray(np.asarray(V, dtype=np.float16))
    W = np.ascontiguousarray(np.asarray(W, dtype=np.float16))
    # per-chunk d-major transposes: [B_LOC, E, D] -> [N_CHUNKS, D, E, CHUNK]
    prevT = np.ascontiguousarray(
        prev.reshape(N_CORES, N_CHUNKS, CHUNK, E, D).transpose(0, 1, 4, 3, 2)
    )
    keysT = np.ascontiguousarray(
        kys.reshape(N_CORES, N_CHUNKS, CHUNK, E, D).transpose(0, 1, 4, 3, 2)
    )
    encT = np.ascontiguousarray(
        enc.reshape(N_CORES, N_CHUNKS, CHUNK, D).transpose(0, 1, 3, 2)
    )
    prev_row = np.ascontiguousarray(prev)
    in_maps = []
    for i in range(N_CORES):
        lo, hi = i * B_LOC, (i + 1) * B_LOC
        in_maps.append(
            {
                "prevT": prevT[i],
                "keysT": keysT[i],
                "encT": encT[i],
                "prev": prev_row[lo:hi],
                "U": U,
                "V": V,
                "W": W,
            }
        )
    return in_maps


def kernel(encoded_sents, prev_states, keys, U, V, W):
    import sys

    if "/opt/trn_rl_repo" not in sys.path:
        sys.path.insert(0, "/opt/trn_rl_repo")
    from concourse.bass_utils import run_bass_kernel_spmd

    nc = _get_nc()
    in_maps = make_in_maps(encoded_sents, prev_states, keys, U, V, W)
    res = run_bass_kernel_spmd(nc, in_maps, list(range(N_CORES)))
    out = np.concatenate([res.results[i]["out"] for i in range(N_CORES)], axis=0)
    return out.astype(np.float32)
